# revision 11
# baseline (speedup 1.0000x reference)
"""Correlation cost-volume kernel for Trainium2 (8 NeuronCores).

out[b, d, y, x] = mean_c in1[b,c,y,x] * pad(in2)[b,c,y+dy,x+dx],
d = (dy+4)*9 + (dx+4), 81 displacements.

Sharding: pure data parallel over batch (B=8 -> 1 batch element per core).

Per core: both inputs live in SBUF whole, as bf16, column-major ([c, x, y]);
in1 is pre-scaled by 1/C on the host (exact exponent shift in bf16).

For each output column x (256 of them):
  - 4 col-tiled matmuls (tile_position=(0,32t)): stationary = in1T[:, x, 32t:32t+32]
    (128c x 32y), moving = in2T[:, x:x+9, 32t:32t+40] as 360 columns ordered
    (y' outer, dx inner) -> PSUM P[y, j]. For partition y = 32t+u the 81 band
    values land CONTIGUOUSLY at j in [9u, 9u+81), already in d-order.
  - Groups of GX=4 x share one 4-bank PSUM tile; one DVE/ACT copy (alternating
    engines) evacuates PSUM -> bf16 SBUF stage (PSUM reads are 1x; a single big
    copy amortizes the per-instruction bubble).
  - Stages hold GS=16 x; 32 per-u band DMAs per stage (pure-partition strides)
    write the compact [4t, 16x, 81d] slices: 5.3 MB output traffic vs 85 MB.

Host side: transpose/cast inputs, final [x,y,d] -> [d,y,x] permute + f32 cast.

The toolchain rejects instructions with >1 sync wait, so after tracing we
split extra waits onto same-engine NoOps (split_multi_waits).
"""
import numpy as np

B, C, H, W = 8, 128, 128, 256
PAD = 4
ND = 9             # displacements per axis
NDISP = ND * ND    # 81
HP = H + 2 * PAD   # 136 padded column height
NH = W + 2 * PAD   # 264 padded row width
SW = 40 * ND       # 360 psum stream width per x-column
GX = 4             # x-columns per psum tile (4 banks)
GS = 32            # x-columns per stage tile

_CACHE = {}


def _build(split_waits: bool = True, sim_mode: bool = False):
    import concourse.bass as bass
    import concourse.mybir as mybir
    import bass_rust
    from concourse.ap import AP
    from concourse.tile import TileContext

    f32 = mybir.dt.float32
    bf16 = mybir.dt.bfloat16

    nc = bass.Bass()
    # in1t[c, x*H + y] = in1[c, y, x] / C     (bf16)
    IN1T = nc.dram_tensor("in1t", [C, W * H], bf16, kind="ExternalInput")
    # in2t[c, xi*HP + yi] = pad(in2)[c, yi, xi]  (bf16)
    IN2T = nc.dram_tensor("in2t", [C, NH * HP], bf16, kind="ExternalInput")
    # outd[x, t, u, d] = out[d, 32t+u, x]   (bf16)
    OUT = nc.dram_tensor("outd", [W, 4, 32, NDISP], bf16, kind="ExternalOutput")
    OP_T = 32 * NDISP          # 2592
    OP_X = 4 * OP_T            # 10368
    if sim_mode:
        # sim's AP checker can't view the per-u band APs; dump raw stages
        OUTF = nc.dram_tensor("outf", [W // GS, 128, GS * SW], bf16,
                              kind="ExternalOutput")

    with TileContext(nc) as tc:
        with tc.tile_pool(name="pin", bufs=1) as pin, \
             tc.tile_pool(name="ppsum", bufs=2, space="PSUM") as ppsum, \
             tc.tile_pool(name="pstage", bufs=2) as pstage:
            t1 = pin.tile([C, W * H], bf16)
            t2 = pin.tile([C, NH * HP], bf16)
            NCH = 8
            for i in range(NCH):
                c0 = i * (W // NCH) * H
                c1 = (i + 1) * (W // NCH) * H
                nc.gpsimd.dma_start(out=t1[:, c0:c1], in_=IN1T[:, c0:c1])
            for i in range(NCH):
                c0 = i * (NH // NCH) * HP
                c1 = (i + 1) * (NH // NCH) * HP
                nc.gpsimd.dma_start(out=t2[:, c0:c1], in_=IN2T[:, c0:c1])
            t1t, t1o = t1[:].tensor, t1[:].offset
            t2t, t2o = t2[:].tensor, t2[:].offset
            P1 = t1[:].ap[0][0]
            P2 = t2[:].ap[0][0]

            for s in range(W // GS):
                S = pstage.tile([128, GS * SW], bf16)
                st, so = S[:].tensor, S[:].offset
                SP = S[:].ap[0][0]
                for gg in range(GS // GX):
                    g = s * (GS // GX) + gg
                    x0 = GX * g
                    P = ppsum.tile([128, 2048], f32)
                    PP = P[:].ap[0][0]
                    for xi in range(GX):
                        x = x0 + xi
                        for t in range(4):
                            lhsT = AP(t1t, t1o + x * H + 32 * t,
                                      [[P1, C], [1, 32]])
                            rhs = AP(t2t, t2o + x * HP + 32 * t,
                                     [[P2, C], [1, 40], [HP, ND]])
                            nc.tensor.matmul(
                                P[32 * t:32 * t + 32, 512 * xi:512 * xi + SW],
                                lhsT, rhs, start=True, stop=True,
                                tile_position=(0, 32 * t))
                    pt, po = P[:].tensor, P[:].offset
                    csrc = AP(pt, po, [[PP, 128], [512, GX], [1, SW]])
                    cdst = AP(st, so + gg * GX * SW,
                              [[SP, 128], [SW, GX], [1, SW]])
                    if g % 2 == 0:
                        nc.scalar.copy(cdst, csrc)
                    else:
                        nc.vector.tensor_scalar_mul(cdst, csrc, 1.0)
                # 32 per-u band dumps (partition strides pure, cols in-bounds)
                xs0 = s * GS
                if sim_mode:
                    nc.sync.dma_start(out=OUTF[s], in_=S[:])
                else:
                    for u in range(32):
                        dsrc = AP(st, so + u * SP + 9 * u,
                                  [[32 * SP, 4], [SW, GS], [1, NDISP]])
                        ddst = AP(OUT[:].tensor, xs0 * OP_X + u * NDISP,
                                  [[OP_T, 4], [OP_X, GS], [1, NDISP]])
                        eng = nc.sync if u % 2 == 0 else nc.scalar
                        eng.dma_start(out=ddst, in_=dsrc)

    # --- split multi-wait instructions (this walrus accepts max 1) ---
    if not split_waits:
        return nc
    n = 0
    for fn in nc.m.functions:
        for blk in fn.blocks:
            il = blk.instructions
            new = []
            changed = False
            for ins in il:
                si = ins.sync_info
                if si is not None and len(si.on_wait) > 1:
                    waits = list(si.on_wait)
                    for w in waits[:-1]:
                        n += 1
                        new.append(bass_rust.InstNoOp(
                            name=f"wsplit_{n}", engine=ins.engine,
                            sync_info=bass_rust.SyncInfo(
                                on_wait=[w], on_update=[])))
                    si.on_wait = waits[-1:]
                    ins.sync_info = si
                    changed = True
                new.append(ins)
            if changed:
                blk.instructions = new
    return nc


def _get_nc():
    if "nc" not in _CACHE:
        _CACHE["nc"] = _build()
    return _CACHE["nc"]


def _prep_core(in1_b: np.ndarray, in2_b: np.ndarray) -> dict:
    import ml_dtypes
    bf = ml_dtypes.bfloat16
    in1t = (np.asarray(in1_b).transpose(0, 2, 1) * np.float32(1.0 / C)).astype(bf)
    in2p = np.pad(np.asarray(in2_b), ((0, 0), (PAD, PAD), (PAD, PAD)))
    in2t = in2p.transpose(0, 2, 1).astype(bf)
    return {"in1t": np.ascontiguousarray(in1t).reshape(C, W * H),
            "in2t": np.ascontiguousarray(in2t).reshape(C, NH * HP)}


def kernel(input1: np.ndarray, input2: np.ndarray) -> np.ndarray:
    from concourse.bass_utils import run_bass_kernel_spmd

    input1 = np.ascontiguousarray(input1, dtype=np.float32)
    input2 = np.ascontiguousarray(input2, dtype=np.float32)
    in_maps = [_prep_core(input1[b], input2[b]) for b in range(B)]

    nc = _get_nc()
    results = run_bass_kernel_spmd(nc, in_maps, core_ids=list(range(B))).results

    out = np.empty((B, NDISP, H, W), dtype=np.float32)
    for b in range(B):
        D = results[b]["outd"].astype(np.float32)    # [W, 4, 32, 81]
        out[b] = D.reshape(W, H, NDISP).transpose(2, 1, 0)
    return out


# revision 15
# speedup vs baseline: 1.6614x; 1.6614x over previous
"""Correlation cost-volume kernel for Trainium2 (8 NeuronCores).

out[b, d, y, x] = mean_c in1[b,c,y,x] * pad(in2)[b,c,y+dy,x+dx],
d = (dy+4)*9 + (dx+4), 81 displacements.

Sharding: pure data parallel over batch (B=8 -> 1 batch element per core).

Per core: both inputs live in SBUF whole, as bf16, column-major ([c, x, y]);
in1 is pre-scaled by 1/C on the host (exact exponent shift in bf16).

For each output column x (256 of them):
  - 4 col-tiled matmuls (tile_position=(0,32t)): stationary = in1T[:, x, 32t:32t+32]
    (128c x 32y), moving = in2T[:, x:x+9, 32t:32t+40] as 360 columns ordered
    (y' outer, dx inner) -> PSUM P[y, j]. For partition y = 32t+u the 81 band
    values land CONTIGUOUSLY at j in [9u, 9u+81), already in d-order.
  - Groups of GX=4 x share one 4-bank PSUM tile; one DVE/ACT copy (alternating
    engines) evacuates PSUM -> bf16 SBUF stage (PSUM reads are 1x; a single big
    copy amortizes the per-instruction bubble).
  - Stages hold GS=16 x; 32 per-u band DMAs per stage (pure-partition strides)
    write the compact [4t, 16x, 81d] slices: 5.3 MB output traffic vs 85 MB.

Host side: transpose/cast inputs, final [x,y,d] -> [d,y,x] permute + f32 cast.

The toolchain rejects instructions with >1 sync wait, so after tracing we
split extra waits onto same-engine NoOps (split_multi_waits).
"""
import numpy as np

B, C, H, W = 8, 128, 128, 256
PAD = 4
ND = 9             # displacements per axis
NDISP = ND * ND    # 81
HP = H + 2 * PAD   # 136 padded column height
NH = W + 2 * PAD   # 264 padded row width
SW = 40 * ND       # 360 psum stream width per x-column
GX = 4             # x-columns per psum tile (4 banks)
GS = 32            # x-columns per stage tile

_CACHE = {}


def _build(split_waits: bool = True, sim_mode: bool = False):
    import concourse.bass as bass
    import concourse.mybir as mybir
    import bass_rust
    from concourse.ap import AP
    from concourse.tile import TileContext

    f32 = mybir.dt.float32
    bf16 = mybir.dt.bfloat16

    nc = bass.Bass()
    # in1t[c, x*H + y] = in1[c, y, x] / C     (bf16)
    IN1T = nc.dram_tensor("in1t", [C, W * H], bf16, kind="ExternalInput")
    # in2t[c, xi*HP + yi] = pad(in2)[c, yi, xi]  (bf16)
    IN2T = nc.dram_tensor("in2t", [C, NH * HP], bf16, kind="ExternalInput")
    # outd[x, t, u, d] = out[d, 32t+u, x]   (bf16)
    # raw stage dump: outf[s, y, 360*xi + 9*(y%32) + d] holds the bands;
    # the host extracts them (a DMA-side band gather would need per-partition
    # offsets, which neither HWDGE descriptors nor engine APs can express
    # without exploding into 1us-per-trigger descriptor generation).
    OUTF = nc.dram_tensor("outf", [W // GS, 128, GS * SW], bf16,
                          kind="ExternalOutput")

    with TileContext(nc) as tc:
        with tc.tile_pool(name="pin", bufs=1) as pin, \
             tc.tile_pool(name="ppsum", bufs=2, space="PSUM") as ppsum, \
             tc.tile_pool(name="pstage", bufs=2) as pstage:
            t1 = pin.tile([C, W * H], bf16)
            t2 = pin.tile([C, NH * HP], bf16)
            NCH = 8
            for i in range(NCH):
                c0 = i * (W // NCH) * H
                c1 = (i + 1) * (W // NCH) * H
                nc.gpsimd.dma_start(out=t1[:, c0:c1], in_=IN1T[:, c0:c1])
            for i in range(NCH):
                c0 = i * (NH // NCH) * HP
                c1 = (i + 1) * (NH // NCH) * HP
                nc.gpsimd.dma_start(out=t2[:, c0:c1], in_=IN2T[:, c0:c1])
            t1t, t1o = t1[:].tensor, t1[:].offset
            t2t, t2o = t2[:].tensor, t2[:].offset
            P1 = t1[:].ap[0][0]
            P2 = t2[:].ap[0][0]

            for s in range(W // GS):
                S = pstage.tile([128, GS * SW], bf16)
                st, so = S[:].tensor, S[:].offset
                SP = S[:].ap[0][0]
                assert SP == GS * SW
                for gg in range(GS // GX):
                    g = s * (GS // GX) + gg
                    x0 = GX * g
                    P = ppsum.tile([128, 2048], f32)
                    PP = P[:].ap[0][0]
                    for xi in range(GX):
                        x = x0 + xi
                        for t in range(4):
                            lhsT = AP(t1t, t1o + x * H + 32 * t,
                                      [[P1, C], [1, 32]])
                            rhs = AP(t2t, t2o + x * HP + 32 * t,
                                     [[P2, C], [1, 40], [HP, ND]])
                            nc.tensor.matmul(
                                P[32 * t:32 * t + 32, 512 * xi:512 * xi + SW],
                                lhsT, rhs, start=True, stop=True,
                                tile_position=(0, 32 * t))
                    pt, po = P[:].tensor, P[:].offset
                    csrc = AP(pt, po, [[PP, 128], [512, GX], [1, SW]])
                    cdst = AP(st, so + gg * GX * SW,
                              [[SP, 128], [SW, GX], [1, SW]])
                    if g % 2 == 0:
                        nc.scalar.copy(cdst, csrc)
                    else:
                        nc.vector.tensor_scalar_mul(cdst, csrc, 1.0)
                nc.sync.dma_start(out=OUTF[s], in_=S[:])

    # --- split multi-wait instructions (this walrus accepts max 1) ---
    if not split_waits:
        return nc
    n = 0
    for fn in nc.m.functions:
        for blk in fn.blocks:
            il = blk.instructions
            new = []
            changed = False
            for ins in il:
                si = ins.sync_info
                if si is not None and len(si.on_wait) > 1:
                    waits = list(si.on_wait)
                    for w in waits[:-1]:
                        n += 1
                        new.append(bass_rust.InstNoOp(
                            name=f"wsplit_{n}", engine=ins.engine,
                            sync_info=bass_rust.SyncInfo(
                                on_wait=[w], on_update=[])))
                    si.on_wait = waits[-1:]
                    ins.sync_info = si
                    changed = True
                new.append(ins)
            if changed:
                blk.instructions = new
    return nc


def _get_nc():
    if "nc" not in _CACHE:
        _CACHE["nc"] = _build()
    return _CACHE["nc"]


def _prep_core(in1_b: np.ndarray, in2_b: np.ndarray) -> dict:
    import ml_dtypes
    bf = ml_dtypes.bfloat16
    in1t = (np.asarray(in1_b).transpose(0, 2, 1) * np.float32(1.0 / C)).astype(bf)
    in2p = np.pad(np.asarray(in2_b), ((0, 0), (PAD, PAD), (PAD, PAD)))
    in2t = in2p.transpose(0, 2, 1).astype(bf)
    return {"in1t": np.ascontiguousarray(in1t).reshape(C, W * H),
            "in2t": np.ascontiguousarray(in2t).reshape(C, NH * HP)}


def kernel(input1: np.ndarray, input2: np.ndarray) -> np.ndarray:
    from concourse.bass_utils import run_bass_kernel_spmd

    input1 = np.ascontiguousarray(input1, dtype=np.float32)
    input2 = np.ascontiguousarray(input2, dtype=np.float32)
    in_maps = [_prep_core(input1[b], input2[b]) for b in range(B)]

    nc = _get_nc()
    results = run_bass_kernel_spmd(nc, in_maps, core_ids=list(range(B))).results

    # band gather: out[d, y, 32s+xi] = F[s, y, 360*xi + 9*(y%32) + d]
    J = (360 * np.arange(GS)[None, :, None]
         + 9 * (np.arange(H)[:, None, None] % 32)
         + np.arange(NDISP)[None, None, :])         # [H, GS, 81]
    Jf = J.reshape(H, GS * NDISP)
    out = np.empty((B, NDISP, H, W), dtype=np.float32)
    for b in range(B):
        F = results[b]["outf"]                       # [W//GS, H, GS*SW] bf16
        for s in range(W // GS):
            band = np.take_along_axis(F[s], Jf, axis=1)
            band = band.reshape(H, GS, NDISP).astype(np.float32)
            out[b, :, :, GS * s:GS * s + GS] = band.transpose(2, 0, 1)
    return out


# revision 17
# speedup vs baseline: 2.1927x; 1.3197x over previous
"""Correlation cost-volume kernel v4: 8x8 pixel blocks, 2 blocks per PSUM bank.

out[b, d, y, x] = mean_c in1[b,c,y,x] * pad(in2)[b,c,y+dy,x+dx], d=(dy+4)*9+(dx+4)

Per core (1 batch element):
  - inputs resident in SBUF as bf16 column-major; in1 pre-scaled by 1/C.
  - block = 8y x 8x = 64 pixels; one matmul per block:
      stationary = in1T[:, x0:x0+8, y0:y0+8] (128c x 64 pixels),
      moving     = in2T[:, x0:x0+16, y0:y0+16] as 256 cols (v=yi outer, w=xi inner)
      -> PSUM[64 pixels, 256] where pixel (uy,ux) band lives at
         col (uy+dy+4)*16 + (ux+dx+4).
  - 2 blocks (x-adjacent) share a PSUM bank via col-tiling (tile_position (0,0)/(0,64));
    4 pairs share a 4-bank psum tile; one DVE/ACT copy (alternating) evacuates
    each psum tile to a bf16 stage; one contiguous DMA per 8 psum tiles dumps
    the stage (big descriptors). Host extracts the 9x9 bands (strided view).

Streamed columns per pixel = 256/64 = 4 (vs 11.25 for 32x1 col tiles):
PE ~77us; writes 16.8 MB; reads 17.6 MB.
"""
import numpy as np

B, C, H, W = 8, 128, 128, 256
PAD = 4
ND = 9
NDISP = ND * ND
HP = H + 2 * PAD   # 136
NH = W + 2 * PAD   # 264
BY, BX = 8, 8      # block pixel dims
SBW = 16 * 16      # streamed cols per block: 256
NYB = H // BY      # 16 y-blocks
NXP = W // (2 * BX)  # 16 x-pairs
NQ = NYB * NXP     # 256 pairs
QT = 8             # pairs per psum tile (2 per bank; blocks share cols,
                   # split by partition half like standard col-tiling)
TS = 2             # psum tiles per stage

_CACHE = {}


def _build(split_waits: bool = True):
    import concourse.bass as bass
    import concourse.mybir as mybir
    import bass_rust
    from concourse.ap import AP
    from concourse.tile import TileContext

    f32 = mybir.dt.float32
    bf16 = mybir.dt.bfloat16

    nc = bass.Bass()
    # in1t block-major: in1t[c, ((xb*16+yb)*8+uy)*8+ux] = in1[c,8yb+uy,8xb+ux]/C
    # (walrus requires a 2D weights AP, so each block's 64 stationary columns
    # must be contiguous)
    IN1T = nc.dram_tensor("in1t", [C, W * H], bf16, kind="ExternalInput")
    IN2T = nc.dram_tensor("in2t", [C, NH * HP], bf16, kind="ExternalInput")
    # outf[s, p, 256*yb + 16*v + w]; pair q = 16*xp + yb, xp = s = q//16
    OUTF = nc.dram_tensor("outf", [NQ // (QT * TS), 128, QT * TS * 256], bf16,
                          kind="ExternalOutput")

    with TileContext(nc) as tc:
        with tc.tile_pool(name="pin", bufs=1) as pin, \
             tc.tile_pool(name="ppsum", bufs=2, space="PSUM") as ppsum, \
             tc.tile_pool(name="pstage", bufs=2) as pstage:
            t1 = pin.tile([C, W * H], bf16)
            t2 = pin.tile([C, NH * HP], bf16)
            NCH = 8
            for i in range(NCH):
                c0 = i * (W // NCH) * H
                c1 = (i + 1) * (W // NCH) * H
                nc.gpsimd.dma_start(out=t1[:, c0:c1], in_=IN1T[:, c0:c1])
            for i in range(NCH):
                c0 = i * (NH // NCH) * HP
                c1 = (i + 1) * (NH // NCH) * HP
                nc.gpsimd.dma_start(out=t2[:, c0:c1], in_=IN2T[:, c0:c1])
            t1t, t1o = t1[:].tensor, t1[:].offset
            t2t, t2o = t2[:].tensor, t2[:].offset
            P1 = t1[:].ap[0][0]
            P2 = t2[:].ap[0][0]

            for s in range(NQ // (QT * TS)):        # 16 stages (= x-pairs)
                S = pstage.tile([128, QT * TS * 256], bf16)
                st, so = S[:].tensor, S[:].offset
                SP = S[:].ap[0][0]
                assert SP == QT * TS * 256
                for ti in range(TS):                # 2 psum tiles per stage
                    P = ppsum.tile([128, 2048], f32)
                    PP = P[:].ap[0][0]
                    for pp in range(QT):            # 8 pairs per psum tile
                        q = (s * TS + ti) * QT + pp
                        xp, yb = q // NYB, q % NYB
                        y0 = BY * yb
                        for b in range(2):          # 2 blocks per pair
                            x0 = 16 * xp + 8 * b
                            xb = 2 * xp + b
                            lhsT = AP(t1t, t1o + (xb * NYB + yb) * 64,
                                      [[P1, C], [1, 64]])
                            rhs = AP(t2t, t2o + x0 * HP + y0,
                                     [[P2, C], [1, 16], [HP, 16]])
                            # two pairs share each 2KB bank; start=True lazily
                            # zeroes the whole bank (per partition), so only
                            # the first pair per bank may set it
                            nc.tensor.matmul(
                                P[64 * b:64 * b + 64, 256 * pp:256 * pp + 256],
                                lhsT, rhs, start=(pp % 2 == 0), stop=True,
                                skip_group_check=True,
                                tile_position=(0, 64 * b))
                    pt, po = P[:].tensor, P[:].offset
                    csrc = AP(pt, po, [[PP, 128], [1, 2048]])
                    cdst = AP(st, so + ti * 2048, [[SP, 128], [1, 2048]])
                    if ti % 2 == 0:
                        nc.scalar.copy(cdst, csrc)
                    else:
                        nc.vector.tensor_scalar_mul(cdst, csrc, 1.0)
                nc.sync.dma_start(out=OUTF[s], in_=S[:])

    if not split_waits:
        return nc
    n = 0
    for fn in nc.m.functions:
        for blk in fn.blocks:
            il = blk.instructions
            new = []
            changed = False
            for ins in il:
                si = ins.sync_info
                if si is not None and len(si.on_wait) > 1:
                    waits = list(si.on_wait)
                    for w in waits[:-1]:
                        n += 1
                        new.append(bass_rust.InstNoOp(
                            name=f"wsplit_{n}", engine=ins.engine,
                            sync_info=bass_rust.SyncInfo(
                                on_wait=[w], on_update=[])))
                    si.on_wait = waits[-1:]
                    ins.sync_info = si
                    changed = True
                new.append(ins)
            if changed:
                blk.instructions = new
    return nc


def _get_nc():
    if "nc" not in _CACHE:
        _CACHE["nc"] = _build()
    return _CACHE["nc"]


def _prep_core(in1_b: np.ndarray, in2_b: np.ndarray) -> dict:
    import ml_dtypes
    bf = ml_dtypes.bfloat16
    in1s = (np.asarray(in1_b) * np.float32(1.0 / C)).astype(bf)
    # [c, y, x] -> [c, xb, yb, uy, ux]
    in1t = in1s.reshape(C, NYB, BY, W // BX, BX).transpose(0, 3, 1, 2, 4)
    in2p = np.pad(np.asarray(in2_b), ((0, 0), (PAD, PAD), (PAD, PAD)))
    in2t = in2p.transpose(0, 2, 1).astype(bf)
    return {"in1t": np.ascontiguousarray(in1t).reshape(C, W * H),
            "in2t": np.ascontiguousarray(in2t).reshape(C, NH * HP)}


def _extract(F: np.ndarray) -> np.ndarray:
    """F: [16, 128, 4096] bf16 -> out [81, H, W] f32."""
    from numpy.lib.stride_tricks import as_strided
    out = np.empty((NDISP, H, W), dtype=np.float32)
    Ff = F.astype(np.float32)
    for s in range(NQ // (QT * TS)):
        for r in range(QT * TS):
            q = s * (QT * TS) + r
            xp, yb = q // NYB, q % NYB
            sub = Ff[s][:, 256 * r:256 * r + 256]      # [128, 256]
            for b in range(2):
                blk = sub[64 * b:64 * b + 64, :]
                blk = np.ascontiguousarray(blk).reshape(BY, BX, 16, 16)
                s0, s1, s2, s3 = blk.strides
                # band[uy,ux,dyi,dxi] = blk[uy,ux,uy+dyi,ux+dxi], dyi = dy+4
                band = as_strided(blk, (BY, BX, ND, ND),
                                  (s0 + s2, s1 + s3, s2, s3))
                # band[uy, ux, dy, dx] -> out[dy*9+dx, 8yb+uy, 16xp+8b+ux]
                out[:, BY * yb:BY * yb + BY,
                    16 * xp + 8 * b:16 * xp + 8 * b + BX] = \
                    band.transpose(2, 3, 0, 1).reshape(NDISP, BY, BX)
    return out


def kernel(input1: np.ndarray, input2: np.ndarray) -> np.ndarray:
    from concourse.bass_utils import run_bass_kernel_spmd

    input1 = np.ascontiguousarray(input1, dtype=np.float32)
    input2 = np.ascontiguousarray(input2, dtype=np.float32)
    in_maps = [_prep_core(input1[b], input2[b]) for b in range(B)]

    nc = _get_nc()
    results = run_bass_kernel_spmd(nc, in_maps, core_ids=list(range(B))).results

    out = np.empty((B, NDISP, H, W), dtype=np.float32)
    for b in range(B):
        out[b] = _extract(results[b]["outf"])
    return out
